# revision 8
# baseline (speedup 1.0000x reference)
"""GQA causal-attention prefill kernel for 8 Trainium2 NeuronCores.

Sharding: core c -> (batch b = c//4, kv head g = c%4).
Replica groups [[0,1,2,3],[4,5,6,7]] (one per batch).

Per-core pipeline (everything feature-major / "transposed" so the token dim
is always the matmul moving dim, full fp32r rate):
  1. q^T/k^T/v^T projections (+bias) from x^T, contraction over D=3584
  2. RoPE on q (7 heads) and k, in-place on DVE
  3. v^T -> v (natural) via PE transposes (P@V needs keys on partitions)
  4. per head: S^T = k^T-chunk.T @ q^T (causal chunks only), +tri-mask on
     diagonal chunks, exp on ACT (no max subtraction: |logits| is small),
     denominator = ones.T @ E^T on PE, O^T_unnorm = v-chunk.T @ E^T,
     normalize via reciprocal + PE outer-product broadcast
  5. AllGather O^T blocks across the 4 cores of the batch
  6. o_proj: this core's 896-column block of y from the full O^T
Output per core: y[b][:, 896g:896(g+1)].T, host concatenates + transposes.
"""
import sys

if '/opt/trn_rl_repo' not in sys.path:
    sys.path.insert(0, '/opt/trn_rl_repo')

import numpy as np

B, T, D = 2, 1024, 3584
NUM_HEADS, HEAD_DIM, NUM_KV = 28, 128, 4
REP = NUM_HEADS // NUM_KV            # 7
KVD = NUM_KV * HEAD_DIM              # 512
ROPE_THETA = 1000000.0
K_MASK = -3.3895313892515355e+38     # bf16 finfo min, as in the reference
SCALE = HEAD_DIM ** -0.5
GROUP = 4                            # tensor-parallel group size (kv heads)
NCORES = 8
DK = D // 128                        # 28 contraction chunks over D
MH = REP                             # 7 head chunks per core (896 = 7*128)
NT = T // 512                        # token 512-tiles
SK = T // 128                        # key 128-chunks

_CACHE = {}


def _build_nc():
    """Build the SPMD Bass program (same program on all 8 cores)."""
    import concourse.bass as bass
    import concourse.tile as tile
    from concourse import bacc, mybir
    from concourse.masks import make_identity

    FP32 = mybir.dt.float32
    FP32R = mybir.dt.float32r
    Exp = mybir.ActivationFunctionType.Exp
    Ident = mybir.ActivationFunctionType.Identity

    nc = bacc.Bacc("TRN2", target_bir_lowering=False, debug=False, num_devices=NCORES)

    xt = nc.dram_tensor("xt", [D, T], FP32R, kind="ExternalInput")
    wq = nc.dram_tensor("wq", [D, REP * 128], FP32R, kind="ExternalInput")
    wk = nc.dram_tensor("wk", [D, 128], FP32R, kind="ExternalInput")
    wv = nc.dram_tensor("wv", [D, 128], FP32R, kind="ExternalInput")
    wo = nc.dram_tensor("wo", [D, REP * 128], FP32R, kind="ExternalInput")
    bqkv = nc.dram_tensor("bqkv", [REP + 2, 128], FP32, kind="ExternalInput")
    sincat = nc.dram_tensor("sincat", [128, T], FP32, kind="ExternalInput")
    coscat = nc.dram_tensor("coscat", [128, T], FP32, kind="ExternalInput")
    trimask = nc.dram_tensor("trimask", [128, 128], FP32, kind="ExternalInput")
    onescol = nc.dram_tensor("onescol", [128, 1], FP32R, kind="ExternalInput")
    yt = nc.dram_tensor("yt", [REP * 128, T], FP32, kind="ExternalOutput")

    mult = mybir.AluOpType.mult
    addop = mybir.AluOpType.add

    with tile.TileContext(nc) as tc:
        with (
            tc.tile_pool(name="consts", bufs=1) as consts,
            tc.tile_pool(name="qkv", bufs=1) as qkv,
            tc.tile_pool(name="dram", bufs=1, space="DRAM") as dram,
        ):
            sin_sb = consts.tile([128, T], FP32, tag="sin")
            cos_sb = consts.tile([128, T], FP32, tag="cos")
            tri_sb = consts.tile([128, 128], FP32, tag="tri")
            id_sb = consts.tile([128, 128], FP32, tag="ident")
            ones_col = consts.tile([128, 1], FP32R, tag="onescol")
            ones_row = consts.tile([1, 128], FP32, tag="onesrow")
            bias_sb = consts.tile([128, REP + 2], FP32, tag="bias")
            nc.sync.dma_start(sin_sb[:], sincat[:])
            nc.sync.dma_start(cos_sb[:], coscat[:])
            nc.sync.dma_start(tri_sb[:], trimask[:])
            nc.sync.dma_start(bias_sb[:], bqkv.rearrange("m p -> p m"))
            nc.sync.dma_start(ones_col[:], onescol[:])
            make_identity(nc, id_sb[:])
            nc.vector.memset(ones_row[:], 1.0)

            q_sb = qkv.tile([128, REP, T], FP32R, tag="q")
            k_sb = qkv.tile([128, T], FP32R, tag="k")
            v_sb = qkv.tile([128, T], FP32, tag="v")
            vn_sb = qkv.tile([128, SK, 128], FP32R, tag="vn")
            oT_sb = qkv.tile([128, REP, T], FP32R, tag="oT")

            # ---- Phase 1: projections, feature-major -------------------
            with (
                tc.tile_pool(name="xp", bufs=1) as xp,
                tc.tile_pool(name="wp", bufs=2) as wp,
                tc.tile_pool(name="pp1", bufs=2, space="PSUM") as pp1,
            ):
                x_sb = xp.tile([128, DK, T], FP32R, tag="x")
                xr = xt.rearrange("(c p) t -> p c t", p=128)
                for cq in range(7):          # 7 DMAs of 4 chunks each
                    nc.sync.dma_start(
                        x_sb[:, 4 * cq:4 * cq + 4, :], xr[:, 4 * cq:4 * cq + 4, :]
                    )

                wqr = wq.rearrange("(c p) n -> p c n", p=128)
                wkr = wk.rearrange("(c p) n -> p c n", p=128)
                wvr = wv.rearrange("(c p) n -> p c n", p=128)

                for m in range(REP + 2):     # 0..6 q heads, 7 = k, 8 = v
                    wtiles = []
                    for quarter in range(4):
                        wt = wp.tile([128, 7, 128], FP32R, tag="w")
                        c0 = 7 * quarter
                        if m < REP:
                            src = wqr[:, c0:c0 + 7, 128 * m:128 * (m + 1)]
                        elif m == REP:
                            src = wkr[:, c0:c0 + 7, :]
                        else:
                            src = wvr[:, c0:c0 + 7, :]
                        nc.sync.dma_start(wt[:], src)
                        wtiles.append(wt)
                    for n in range(NT):
                        ps = pp1.tile([128, 512], FP32, tag="proj")
                        for kc in range(DK):
                            nc.tensor.matmul(
                                ps[:],
                                wtiles[kc // 7][:, kc % 7, :],
                                x_sb[:, kc, 512 * n:512 * (n + 1)],
                                start=(kc == 0),
                                stop=(kc == DK - 1),
                            )
                        if m < REP:
                            dst = q_sb[:, m, 512 * n:512 * (n + 1)]
                        elif m == REP:
                            dst = k_sb[:, 512 * n:512 * (n + 1)]
                        else:
                            dst = v_sb[:, 512 * n:512 * (n + 1)]
                        nc.scalar.activation(
                            dst, ps[:], Ident, bias=bias_sb[:, m:m + 1], scale=1.0
                        )

                # ---- Phase 2: RoPE (in-place), overlaps with P1 tail ---
                with tc.tile_pool(name="ropep", bufs=2) as ropep:
                    targets = [q_sb[:, h, :] for h in range(REP)] + [k_sb[:]]
                    for Xf in targets:
                        for n in range(NT):
                            X = Xf[:, 512 * n:512 * (n + 1)]
                            tmp = ropep.tile([128, 512], FP32, tag="ropetmp")
                            nc.vector.tensor_copy(tmp[0:64, :], X[64:128, :])
                            nc.vector.tensor_copy(tmp[64:128, :], X[0:64, :])
                            sc_sl = (slice(None), slice(512 * n, 512 * (n + 1)))
                            nc.vector.tensor_tensor(tmp[:], tmp[:], sin_sb[sc_sl], op=mult)
                            nc.vector.tensor_tensor(X, X, cos_sb[sc_sl], op=mult)
                            nc.vector.tensor_tensor(X, X, tmp[:], op=addop)

                # ---- Phase 3: v^T -> v natural (PE transposes) ---------
                with tc.tile_pool(name="pptr", bufs=2, space="PSUM") as pptr:
                    for sc in range(SK):
                        tp = pptr.tile([128, 128], FP32, tag="tr")
                        nc.tensor.transpose(
                            tp[:], v_sb[:, 128 * sc:128 * (sc + 1)], id_sb[:]
                        )
                        nc.scalar.copy(vn_sb[:, sc, :], tp[:])

            # ---- Phase 4: attention per head ---------------------------
            og = dram.tile([REP * 128, T], FP32R, tag="og")
            oag = dram.tile([GROUP * REP * 128, T], FP32R, tag="oag")
            ogr = og.rearrange("(h p) t -> p h t", p=128)

            with (
                tc.tile_pool(name="ep", bufs=3) as ep,
                tc.tile_pool(name="ppatt", bufs=2, space="PSUM") as ppatt,
            ):
                for h in range(REP):
                    for tau in range(NT):
                        n_sc = 4 * (tau + 1)
                        den = ppatt.tile([1, 512], FP32, tag="den")
                        ops = ppatt.tile([128, 512], FP32, tag="opv")
                        for c in range(n_sc):
                            delta = 128 * c - 512 * tau
                            t0 = max(delta, 0)
                            w = 512 - t0
                            sps = ppatt.tile([128, 512], FP32, tag="s")
                            tsl = slice(512 * tau + t0, 512 * (tau + 1))
                            nc.tensor.matmul(
                                sps[:, 0:w],
                                k_sb[:, 128 * c:128 * (c + 1)],
                                q_sb[:, h, tsl],
                                start=True,
                                stop=True,
                            )
                            if delta >= 0:
                                nc.vector.tensor_tensor(
                                    sps[:, 0:128], sps[:, 0:128], tri_sb[:], op=addop
                                )
                            et = ep.tile([128, 512], FP32R, tag="e")
                            nc.scalar.activation(et[:, 0:w], sps[:, 0:w], Exp, scale=SCALE)
                            nc.tensor.matmul(
                                den[0:1, t0:512],
                                ones_col[:],
                                et[:, 0:w],
                                start=(c == 0),
                                stop=(c == n_sc - 1),
                            )
                            nc.tensor.matmul(
                                ops[:, t0:512],
                                vn_sb[:, c, :],
                                et[:, 0:w],
                                start=(c == 0),
                                stop=(c == n_sc - 1),
                            )
                        rec = ep.tile([1, 512], FP32, tag="rec")
                        nc.vector.reciprocal(rec[:], den[0:1, :])
                        bc = ppatt.tile([128, 512], FP32, tag="bc")
                        nc.tensor.matmul(bc[:], ones_row[:], rec[:], start=True, stop=True)
                        bcs = ep.tile([128, 512], FP32, tag="bcs")
                        nc.scalar.copy(bcs[:], bc[:])
                        nc.vector.tensor_tensor(
                            oT_sb[:, h, 512 * tau:512 * (tau + 1)], ops[:], bcs[:], op=mult
                        )
                    # ship this head's O^T block to DRAM for the AllGather
                    nc.sync.dma_start(ogr[:, h, :], oT_sb[:, h, :])

            # ---- Phase 5: AllGather O^T across the batch group ---------
            nc.gpsimd.collective_compute(
                "AllGather",
                mybir.AluOpType.bypass,
                replica_groups=[[0, 1, 2, 3], [4, 5, 6, 7]],
                ins=[og[:].opt()],
                outs=[oag[:].opt()],
            )

            # ---- Phase 6: o_proj on this core's 896-column block -------
            with (
                tc.tile_pool(name="otp", bufs=1) as otp,
                tc.tile_pool(name="wp2", bufs=3) as wp2,
                tc.tile_pool(name="outp", bufs=3) as outp,
                tc.tile_pool(name="pp6", bufs=2, space="PSUM") as pp6,
            ):
                otf = otp.tile([128, DK, T], FP32R, tag="otf")
                oagr = oag.rearrange("(c p) t -> p c t", p=128)
                for cq in range(7):
                    nc.sync.dma_start(
                        otf[:, 4 * cq:4 * cq + 4, :], oagr[:, 4 * cq:4 * cq + 4, :]
                    )
                wor = wo.rearrange("(c p) n -> p c n", p=128)
                ytr = yt.rearrange("(m p) t -> p m t", p=128)
                for m in range(MH):
                    wtiles = []
                    for quarter in range(4):
                        wt = wp2.tile([128, 7, 128], FP32R, tag="w2")
                        c0 = 7 * quarter
                        nc.sync.dma_start(
                            wt[:], wor[:, c0:c0 + 7, 128 * m:128 * (m + 1)]
                        )
                        wtiles.append(wt)
                    for n in range(NT):
                        ps = pp6.tile([128, 512], FP32, tag="yps")
                        for kc in range(DK):
                            nc.tensor.matmul(
                                ps[:],
                                wtiles[kc // 7][:, kc % 7, :],
                                otf[:, kc, 512 * n:512 * (n + 1)],
                                start=(kc == 0),
                                stop=(kc == DK - 1),
                            )
                        st = outp.tile([128, 512], FP32, tag="ystage")
                        nc.scalar.copy(st[:], ps[:])
                        nc.sync.dma_start(ytr[:, m, 512 * n:512 * (n + 1)], st[:])

    nc.compile()
    return nc


def _tf32_round(a):
    """Round fp32 -> tf32 (fp32r) representable values, round-to-nearest-even."""
    u = np.ascontiguousarray(a, dtype=np.float32).view(np.uint32)
    u = (u + 0xFFF + ((u >> 13) & 1)) & np.uint32(0xFFFFE000)
    return u.view(np.float32)


def _host_prep(x, segment_ids, Wq, bq, Wk, bk, Wv, bv, Wo):
    """Numpy-side input prep: transpose x, slice weights, RoPE tables, mask."""
    valid = (segment_ids != 0)
    pos = (np.cumsum(valid, axis=-1) - 1).astype(np.int32)  # CUR_IND = 0
    half = HEAD_DIM // 2
    fraction = np.arange(half, dtype=np.float32) / half
    timescale = ROPE_THETA ** fraction
    ang = pos[..., None].astype(np.float32) / timescale      # (B, T, 64)
    sin = np.sin(ang).astype(np.float32)
    cos = np.cos(ang).astype(np.float32)

    # tri[s, t] additive causal mask for a diagonal 128-chunk
    sl = np.arange(128)
    tri = np.where(sl[None, :] >= sl[:, None], 0.0, K_MASK).astype(np.float32)

    in_maps = []
    for c in range(NCORES):
        b, g = c // GROUP, c % GROUP
        qcols = slice(REP * 128 * g, REP * 128 * (g + 1))
        kvcols = slice(128 * g, 128 * (g + 1))
        bias = np.concatenate(
            [bq[qcols].reshape(REP, 128), bk[kvcols][None, :], bv[kvcols][None, :]],
            axis=0,
        ).astype(np.float32)
        sincat = np.concatenate([-sin[b].T, sin[b].T], axis=0)  # (128, T)
        coscat = np.concatenate([cos[b].T, cos[b].T], axis=0)
        in_maps.append({
            "xt": _tf32_round(np.ascontiguousarray(x[b].T, dtype=np.float32)),
            "wq": _tf32_round(np.ascontiguousarray(Wq[:, qcols], dtype=np.float32)),
            "wk": _tf32_round(np.ascontiguousarray(Wk[:, kvcols], dtype=np.float32)),
            "wv": _tf32_round(np.ascontiguousarray(Wv[:, kvcols], dtype=np.float32)),
            "wo": _tf32_round(np.ascontiguousarray(Wo[:, qcols], dtype=np.float32)),
            "bqkv": bias,
            "sincat": np.ascontiguousarray(sincat, dtype=np.float32),
            "coscat": np.ascontiguousarray(coscat, dtype=np.float32),
            "trimask": tri,
            "onescol": np.ones((128, 1), np.float32),
        })
    return in_maps


def _assemble(results):
    y = np.empty((B, T, D), dtype=np.float32)
    for b in range(B):
        blocks = [results[GROUP * b + g]["yt"] for g in range(GROUP)]
        y[b] = np.concatenate(blocks, axis=0).T
    return y


def kernel(x, segment_ids, k_cache, v_cache, Wq, bq, Wk, bk, Wv, bv, Wo,
           _trace=False, _trace_kwargs=None):
    # k_cache/v_cache are zero-initialized and fully overwritten by this
    # prefill (CUR_IND=0, cache_size==T), so they do not affect the output.
    from concourse.bass_utils import run_bass_kernel_spmd

    in_maps = _host_prep(
        np.asarray(x), np.asarray(segment_ids),
        np.asarray(Wq), np.asarray(bq), np.asarray(Wk), np.asarray(bk),
        np.asarray(Wv), np.asarray(bv), np.asarray(Wo),
    )
    if "nc" not in _CACHE:
        _CACHE["nc"] = _build_nc()
    kw = {}
    if _trace:
        kw.update(trace=True, **(_trace_kwargs or {}))
    br = run_bass_kernel_spmd(_CACHE["nc"], in_maps, core_ids=list(range(NCORES)), **kw)
    y = _assemble(br.results)
    if _trace:
        _CACHE["last_result"] = br
    return y


# revision 14
# speedup vs baseline: 1.2204x; 1.2204x over previous
"""GQA causal-attention prefill kernel for 8 Trainium2 NeuronCores.

Sharding: core c -> (batch b = c//4, kv head g = c%4).
Replica groups [[0,1,2,3],[4,5,6,7]] (one per batch).

Per-core pipeline (everything feature-major / "transposed" so the token dim
is always the matmul moving dim, full fp32r rate):
  1. k^T/v^T/q^T projections (+bias) from x^T, contraction over D=3584
  2. RoPE on k then q (7 heads), in-place on DVE
  3. v^T -> v (natural) via PE transposes (P@V needs keys on partitions)
  4. per head: S^T = k^T-chunk.T @ q^T (causal chunks only), +tri-mask on
     diagonal chunks, exp on ACT (no max subtraction: |logits| is small),
     denominator = ones.T @ E^T on PE, O^T_unnorm = v-chunk.T @ E^T,
     normalize via reciprocal + PE outer-product broadcast; per-head
     AllGather of the O^T block overlaps the remaining heads' compute
  5. o_proj: two m-group passes accumulating over heads in AG arrival
     order, so only the last head's gather sits near the critical path
Output per core: y[b][:, 896g:896(g+1)].T, host concatenates + transposes.
"""
import sys

if '/opt/trn_rl_repo' not in sys.path:
    sys.path.insert(0, '/opt/trn_rl_repo')

import numpy as np

B, T, D = 2, 1024, 3584
NUM_HEADS, HEAD_DIM, NUM_KV = 28, 128, 4
REP = NUM_HEADS // NUM_KV            # 7
ROPE_THETA = 1000000.0
K_MASK = -3.3895313892515355e+38     # bf16 finfo min, as in the reference
SCALE = HEAD_DIM ** -0.5
GROUP = 4                            # tensor-parallel group size (kv heads)
NCORES = 8
DK = D // 128                        # 28 contraction chunks over D
NT = T // 512                        # token 512-tiles
SK = T // 128                        # key 128-chunks
RG = [[0, 1, 2, 3], [4, 5, 6, 7]]

_CACHE = {}


def _build_nc():
    """Build the SPMD Bass program (same program on all 8 cores)."""
    import concourse.tile as tile
    from concourse import bacc, mybir
    from concourse.masks import make_identity

    FP32 = mybir.dt.float32
    FP32R = mybir.dt.float32r
    Exp = mybir.ActivationFunctionType.Exp
    Ident = mybir.ActivationFunctionType.Identity
    mult = mybir.AluOpType.mult
    addop = mybir.AluOpType.add

    nc = bacc.Bacc("TRN2", target_bir_lowering=False, debug=False, num_devices=NCORES)

    xt = nc.dram_tensor("xt", [D, T], FP32R, kind="ExternalInput")
    wq = nc.dram_tensor("wq", [D, REP * 128], FP32R, kind="ExternalInput")
    wk = nc.dram_tensor("wk", [D, 128], FP32R, kind="ExternalInput")
    wv = nc.dram_tensor("wv", [D, 128], FP32R, kind="ExternalInput")
    wo = nc.dram_tensor("wo", [D, REP * 128], FP32R, kind="ExternalInput")
    bqkv = nc.dram_tensor("bqkv", [REP + 2, 128], FP32, kind="ExternalInput")
    sincat = nc.dram_tensor("sincat", [128, T], FP32, kind="ExternalInput")
    coscat = nc.dram_tensor("coscat", [128, T], FP32, kind="ExternalInput")
    trimask = nc.dram_tensor("trimask", [128, 128], FP32, kind="ExternalInput")
    onescol = nc.dram_tensor("onescol", [128, 1], FP32R, kind="ExternalInput")
    onesrow = nc.dram_tensor("onesrow", [1, 128], FP32R, kind="ExternalInput")
    yt = nc.dram_tensor("yt", [REP * 128, T], FP32, kind="ExternalOutput")

    with tile.TileContext(nc) as tc:
        with (
            tc.tile_pool(name="consts", bufs=1) as consts,
            tc.tile_pool(name="qkv", bufs=1) as qkv,
            tc.tile_pool(name="dram", bufs=1, space="DRAM") as dram,
            tc.tile_pool(name="ep", bufs=3) as ep,
        ):
            tri_sb = consts.tile([128, 128], FP32, tag="tri")
            ones_col = consts.tile([128, 1], FP32R, tag="onescol")
            ones_row = consts.tile([1, 128], FP32R, tag="onesrow")
            bias_sb = consts.tile([128, REP + 2], FP32, tag="bias")
            nc.sync.dma_start(tri_sb[:], trimask[:])
            nc.sync.dma_start(ones_col[:], onescol[:])
            nc.sync.dma_start(ones_row[:], onesrow[:])
            nc.sync.dma_start(bias_sb[:], bqkv.rearrange("m p -> p m"))

            q_sb = qkv.tile([128, REP, T], FP32R, tag="q")
            k_sb = qkv.tile([128, T], FP32R, tag="k")
            vn_sb = qkv.tile([128, SK, 128], FP32R, tag="vn")

            # per-head DRAM blocks for the pipelined AllGather
            og = [dram.tile([128, T], FP32R, tag=f"og{h}", name=f"og{h}")
                  for h in range(REP)]
            oag = [dram.tile([GROUP * 128, T], FP32R, tag=f"oag{h}", name=f"oag{h}")
                   for h in range(REP)]

            # ---- Phase 1: projections (k, v first, then q heads) --------
            with (
                tc.tile_pool(name="xp", bufs=1) as xp,
                tc.tile_pool(name="wp", bufs=2) as wp,
                tc.tile_pool(name="vp", bufs=1) as vp,
                tc.tile_pool(name="ropep", bufs=2) as ropep,
                tc.tile_pool(name="sincosp", bufs=1) as sincosp,
                tc.tile_pool(name="pp1", bufs=2, space="PSUM") as pp1,
            ):
                sin_sb = sincosp.tile([128, T], FP32, tag="sin")
                cos_sb = sincosp.tile([128, T], FP32, tag="cos")
                nc.sync.dma_start(sin_sb[:], sincat[:])
                nc.sync.dma_start(cos_sb[:], coscat[:])
                id_sb = vp.tile([128, 128], FP32, tag="ident")
                make_identity(nc, id_sb[:])
                v_sb = vp.tile([128, T], FP32, tag="v")

                x_sb = xp.tile([128, DK, T], FP32R, tag="x")
                xr = xt.rearrange("(c p) t -> p c t", p=128)
                for cq in range(7):          # 7 DMAs of 4 chunks each
                    nc.sync.dma_start(
                        x_sb[:, 4 * cq:4 * cq + 4, :], xr[:, 4 * cq:4 * cq + 4, :]
                    )

                wqr = wq.rearrange("(c p) n -> p c n", p=128)
                wkr = wk.rearrange("(c p) n -> p c n", p=128)
                wvr = wv.rearrange("(c p) n -> p c n", p=128)

                def rope(X_full, n):
                    X = X_full[:, 512 * n:512 * (n + 1)]
                    tmp = ropep.tile([128, 512], FP32, tag="ropetmp")
                    nc.vector.tensor_copy(tmp[0:64, :], X[64:128, :])
                    nc.vector.tensor_copy(tmp[64:128, :], X[0:64, :])
                    ssl = (slice(None), slice(512 * n, 512 * (n + 1)))
                    nc.vector.tensor_tensor(tmp[:], tmp[:], sin_sb[ssl], op=mult)
                    nc.vector.tensor_tensor(X, X, cos_sb[ssl], op=mult)
                    nc.vector.tensor_tensor(X, X, tmp[:], op=addop)

                # m: 0 = k, 1 = v, 2.. = q heads 0..6
                for m in range(REP + 2):
                    wtiles = []
                    for quarter in range(4):
                        wt = wp.tile([128, 7, 128], FP32R, tag="w")
                        c0 = 7 * quarter
                        if m == 0:
                            src = wkr[:, c0:c0 + 7, :]
                        elif m == 1:
                            src = wvr[:, c0:c0 + 7, :]
                        else:
                            src = wqr[:, c0:c0 + 7, 128 * (m - 2):128 * (m - 1)]
                        nc.sync.dma_start(wt[:], src)
                        wtiles.append(wt)
                    for n in range(NT):
                        ps = pp1.tile([128, 512], FP32, tag="proj")
                        for kc in range(DK):
                            nc.tensor.matmul(
                                ps[:],
                                wtiles[kc // 7][:, kc % 7, :],
                                x_sb[:, kc, 512 * n:512 * (n + 1)],
                                start=(kc == 0),
                                stop=(kc == DK - 1),
                            )
                        if m == 0:
                            dst, bi = k_sb[:, 512 * n:512 * (n + 1)], 7
                        elif m == 1:
                            dst, bi = v_sb[:, 512 * n:512 * (n + 1)], 8
                        else:
                            dst, bi = q_sb[:, m - 2, 512 * n:512 * (n + 1)], m - 2
                        nc.scalar.activation(
                            dst, ps[:], Ident, bias=bias_sb[:, bi:bi + 1], scale=1.0
                        )
                        if m == 0:
                            rope(k_sb, n)
                        elif m == 1:
                            # v^T chunk -> v natural while v proj streams
                            for sc in range(4 * n, 4 * n + 4):
                                tp = pp1.tile([128, 128], FP32, tag="tr")
                                nc.tensor.transpose(
                                    tp[:], v_sb[:, 128 * sc:128 * (sc + 1)], id_sb[:]
                                )
                                nc.scalar.copy(vn_sb[:, sc, :], tp[:])
                        else:
                            rope(q_sb[:, m - 2, :], n)

            # ---- Phase 4: attention per head + pipelined AllGather ------
            ppatt_ctx = tc.tile_pool(name="ppatt", bufs=1, space="PSUM")
            ppatt = ppatt_ctx.__enter__()
            for h in range(REP):
                for tau in range(NT):
                    n_sc = 4 * (tau + 1)
                    den = ppatt.tile([1, 512], FP32, tag="den")
                    ops = ppatt.tile([128, 512], FP32, tag="opv")
                    for c in range(n_sc):
                        delta = 128 * c - 512 * tau
                        t0 = max(delta, 0)
                        w = 512 - t0
                        sps = ppatt.tile([128, 512], FP32, tag=f"s{c % 2}")
                        tsl = slice(512 * tau + t0, 512 * (tau + 1))
                        nc.tensor.matmul(
                            sps[:, 0:w],
                            k_sb[:, 128 * c:128 * (c + 1)],
                            q_sb[:, h, tsl],
                            start=True,
                            stop=True,
                        )
                        if delta >= 0:
                            nc.vector.tensor_tensor(
                                sps[:, 0:128], sps[:, 0:128], tri_sb[:], op=addop
                            )
                        et = ep.tile([128, 512], FP32R, tag="e")
                        nc.scalar.activation(et[:, 0:w], sps[:, 0:w], Exp, scale=SCALE)
                        nc.tensor.matmul(
                            den[0:1, t0:512], ones_col[:], et[:, 0:w],
                            start=(c == 0), stop=(c == n_sc - 1),
                        )
                        nc.tensor.matmul(
                            ops[:, t0:512], vn_sb[:, c, :], et[:, 0:w],
                            start=(c == 0), stop=(c == n_sc - 1),
                        )
                    rec = ep.tile([1, 512], FP32R, tag="rec")
                    with nc.allow_low_precision(reason="tf32 softmax recip"):
                        nc.vector.reciprocal(rec[:], den[0:1, :])
                    bc = ppatt.tile([128, 512], FP32, tag="den")
                    nc.tensor.matmul(bc[:], ones_row[:], rec[:], start=True, stop=True)
                    bcs = ep.tile([128, 512], FP32, tag="bcs")
                    nc.scalar.copy(bcs[:], bc[:])
                    ost = ep.tile([128, 512], FP32R, tag="ost")
                    nc.vector.tensor_tensor(ost[:], ops[:], bcs[:], op=mult)
                    nc.sync.dma_start(og[h][:, 512 * tau:512 * (tau + 1)], ost[:])
                nc.gpsimd.collective_compute(
                    "AllGather",
                    mybir.AluOpType.bypass,
                    replica_groups=RG,
                    ins=[og[h][:].opt()],
                    outs=[oag[h][:].opt()],
                )

            ppatt_ctx.__exit__(None, None, None)

            # ---- Phase 6: o_proj, consuming AG chunks in arrival order --
            with (
                tc.tile_pool(name="otp", bufs=1) as otp,
                tc.tile_pool(name="wp2", bufs=4) as wp2,
                tc.tile_pool(name="outp", bufs=3) as outp,
                tc.tile_pool(name="pp6", bufs=8, space="PSUM") as pp6,
            ):
                otf = otp.tile([128, DK, T], FP32R, tag="otf")
                for h in range(REP):
                    for gp in range(GROUP):
                        nc.sync.dma_start(
                            otf[:, 7 * gp + h, :],
                            oag[h][128 * gp:128 * (gp + 1), :],
                        )
                wor = wo.rearrange("(c p) n -> p c n", p=128)
                ytr = yt.rearrange("(m p) t -> p m t", p=128)
                for mlist in ([0, 1, 2, 3], [4, 5, 6]):
                    ys = {}
                    for m in mlist:
                        for n in range(NT):
                            ys[m, n] = pp6.tile([128, 512], FP32, tag="y", name=f"y_{m}_{n}")
                    first, last = (0, 0), (REP - 1, GROUP - 1)
                    for h in range(REP):
                        for gp in range(GROUP):
                            hg = 7 * gp + h
                            wt = wp2.tile([128, len(mlist) * 128], FP32R, tag="w2")
                            nc.sync.dma_start(
                                wt[:],
                                wor[:, hg, 128 * mlist[0]:128 * (mlist[-1] + 1)],
                            )
                            for mi, m in enumerate(mlist):
                                for n in range(NT):
                                    nc.tensor.matmul(
                                        ys[m, n][:],
                                        wt[:, 128 * mi:128 * (mi + 1)],
                                        otf[:, hg, 512 * n:512 * (n + 1)],
                                        start=((h, gp) == first),
                                        stop=((h, gp) == last),
                                    )
                    for m in mlist:
                        for n in range(NT):
                            st = outp.tile([128, 512], FP32, tag="ystage")
                            nc.scalar.copy(st[:], ys[m, n][:])
                            nc.sync.dma_start(
                                ytr[:, m, 512 * n:512 * (n + 1)], st[:]
                            )

    nc.compile()
    return nc


def _tf32_round(a):
    """Round fp32 -> tf32 (fp32r) representable values, round-to-nearest-even."""
    u = np.ascontiguousarray(a, dtype=np.float32).view(np.uint32)
    u = (u + 0xFFF + ((u >> 13) & 1)) & np.uint32(0xFFFFE000)
    return u.view(np.float32)


def _host_prep(x, segment_ids, Wq, bq, Wk, bk, Wv, bv, Wo):
    """Numpy-side input prep: transpose x, slice weights, RoPE tables, mask."""
    valid = (segment_ids != 0)
    pos = (np.cumsum(valid, axis=-1) - 1).astype(np.int32)  # CUR_IND = 0
    half = HEAD_DIM // 2
    fraction = np.arange(half, dtype=np.float32) / half
    timescale = ROPE_THETA ** fraction
    ang = pos[..., None].astype(np.float32) / timescale      # (B, T, 64)
    sin = np.sin(ang).astype(np.float32)
    cos = np.cos(ang).astype(np.float32)

    sl = np.arange(128)
    tri = np.where(sl[None, :] >= sl[:, None], 0.0, K_MASK).astype(np.float32)

    in_maps = []
    for c in range(NCORES):
        b, g = c // GROUP, c % GROUP
        qcols = slice(REP * 128 * g, REP * 128 * (g + 1))
        kvcols = slice(128 * g, 128 * (g + 1))
        bias = np.concatenate(
            [bq[qcols].reshape(REP, 128), bk[kvcols][None, :], bv[kvcols][None, :]],
            axis=0,
        ).astype(np.float32)
        sincat = np.concatenate([-sin[b].T, sin[b].T], axis=0)  # (128, T)
        coscat = np.concatenate([cos[b].T, cos[b].T], axis=0)
        in_maps.append({
            "xt": _tf32_round(np.ascontiguousarray(x[b].T, dtype=np.float32)),
            "wq": _tf32_round(np.ascontiguousarray(Wq[:, qcols], dtype=np.float32)),
            "wk": _tf32_round(np.ascontiguousarray(Wk[:, kvcols], dtype=np.float32)),
            "wv": _tf32_round(np.ascontiguousarray(Wv[:, kvcols], dtype=np.float32)),
            "wo": _tf32_round(np.ascontiguousarray(Wo[:, qcols], dtype=np.float32)),
            "bqkv": bias,
            "sincat": np.ascontiguousarray(sincat, dtype=np.float32),
            "coscat": np.ascontiguousarray(coscat, dtype=np.float32),
            "trimask": tri,
            "onescol": np.ones((128, 1), np.float32),
            "onesrow": np.ones((1, 128), np.float32),
        })
    return in_maps


def _assemble(results):
    y = np.empty((B, T, D), dtype=np.float32)
    for b in range(B):
        blocks = [results[GROUP * b + g]["yt"] for g in range(GROUP)]
        y[b] = np.concatenate(blocks, axis=0).T
    return y


def kernel(x, segment_ids, k_cache, v_cache, Wq, bq, Wk, bk, Wv, bv, Wo,
           _trace=False, _trace_kwargs=None):
    # k_cache/v_cache are zero-initialized and fully overwritten by this
    # prefill (CUR_IND=0, cache_size==T), so they do not affect the output.
    from concourse.bass_utils import run_bass_kernel_spmd

    in_maps = _host_prep(
        np.asarray(x), np.asarray(segment_ids),
        np.asarray(Wq), np.asarray(bq), np.asarray(Wk), np.asarray(bk),
        np.asarray(Wv), np.asarray(bv), np.asarray(Wo),
    )
    if "nc" not in _CACHE:
        _CACHE["nc"] = _build_nc()
    kw = {}
    if _trace:
        kw.update(trace=True, **(_trace_kwargs or {}))
    br = run_bass_kernel_spmd(_CACHE["nc"], in_maps, core_ids=list(range(NCORES)), **kw)
    y = _assemble(br.results)
    if _trace:
        _CACHE["last_result"] = br
    return y
